# revision 10
# baseline (speedup 1.0000x reference)
"""Trainium2 Bass kernel: per-batch segment-mean pooling + 3-layer MLP.

Reference computation (B=64, T=512, H=768, S=128):
  pooled[b,s,:] = mean over t of hidden[b,t,:] where statements_ids[b,t]==s
  x = gelu(pooled @ w1 + b1); x = gelu(x @ w2 + b2)
  out[b,s] = sigmoid(x @ w3 + b3)

Distribution: data-parallel over batch across 8 NeuronCores (8 batches per
core); MLP weights replicated.

Per-core algorithm (all matmuls on PE at fp32r 1 cycle/row):
  - Build the one-hot matrix MT[t,s] = (sid[t]==s) on DVE via
    tensor_scalar(is_equal) against an iota constant.
  - counts = MT.T @ ones            (PE)        -> inv = 1/max(counts,1) (DVE)
  - pooled_sums = MT.T @ hidden[b]  (PE, [S,H]) -> pooled = sums*inv     (DVE)
  - X^T tiles via PE transpose (pooled is [S,H] but the MLP wants [H, rows])
  - MLP batched over all 8 local batches: rows = 8*128 = 1024 moving dim,
    weights stationary; gelu/sigmoid + bias fused on ACT.
"""

import os
import sys

sys.path.insert(0, "/opt/trn_rl_repo")

import numpy as np

import concourse.bass as bass
import concourse.mybir as mybir
import concourse.tile as tile
from concourse import bacc, bass_utils

B, T, H, S = 64, 512, 768, 128
N_CORES = 8
BL = B // N_CORES  # local batches per core
P = 128
KT = T // P        # t-tiles per batch
KH = H // P        # h-tiles
R = BL * S         # MLP rows per core
RC = 2 * S         # moving-dim chunk (2 batches) -- >=256 keeps fp32r at 1 cyc/row
NRC = R // RC
HF = H + 2         # hidden padded with 2 constant 1.0 columns (counts trick)

_CACHE: dict = {}


def _build_program(act_func=None):
    f32, f32r, i32 = mybir.dt.float32, mybir.dt.float32r, mybir.dt.int32
    FT = mybir.ActivationFunctionType
    OP = mybir.AluOpType

    nc = bacc.Bacc("TRN2", target_bir_lowering=False, debug=False)
    hid = nc.dram_tensor("hidden", [BL, T, HF], f32r, kind="ExternalInput").ap()
    sid = nc.dram_tensor("sid", [BL, T], i32, kind="ExternalInput").ap()
    w1 = nc.dram_tensor("w1", [H, H], f32r, kind="ExternalInput").ap()
    w2 = nc.dram_tensor("w2", [H, H], f32r, kind="ExternalInput").ap()
    w3 = nc.dram_tensor("w3", [H, 1], f32r, kind="ExternalInput").ap()
    b1 = nc.dram_tensor("b1", [H], f32, kind="ExternalInput").ap()
    b2 = nc.dram_tensor("b2", [H], f32, kind="ExternalInput").ap()
    b3 = nc.dram_tensor("b3", [1], f32, kind="ExternalInput").ap()
    iota = nc.dram_tensor("iota", [P, P], f32, kind="ExternalInput").ap()
    ident = nc.dram_tensor("ident", [P, P], f32, kind="ExternalInput").ap()
    out = nc.dram_tensor("out", [BL, S], f32, kind="ExternalOutput").ap()

    with tile.TileContext(nc) as tc:
        with (
            tc.tile_pool(name="consts", bufs=1) as consts,
            tc.tile_pool(name="wpool", bufs=1) as wpool,
            tc.tile_pool(name="hpool", bufs=1) as hpool,
            tc.tile_pool(name="mtpool", bufs=8) as mtpool,
            tc.tile_pool(name="small", bufs=3) as small,
            tc.tile_pool(name="xtpool", bufs=1) as xtpool,
            tc.tile_pool(name="ypool", bufs=1) as ypool,
            tc.tile_pool(name="ps", bufs=8, space="PSUM") as ps,
        ):
            # ---- constants / small inputs (tiny DMAs first) ----
            iota_sb = consts.tile([P, P], f32)
            nc.sync.dma_start(iota_sb, iota)
            ident_sb = consts.tile([P, P], f32)
            nc.sync.dma_start(ident_sb, ident)
            sid_sb = consts.tile([P, BL, KT], i32)
            nc.sync.dma_start(sid_sb, sid.rearrange("b (k p) -> p b k", p=P))
            b1_sb = consts.tile([P, KH], f32)
            nc.sync.dma_start(b1_sb, b1.rearrange("(m p) -> p m", p=P))
            b2_sb = consts.tile([P, KH], f32)
            nc.sync.dma_start(b2_sb, b2.rearrange("(m p) -> p m", p=P))
            b3_sb = consts.tile([1, 1], f32)
            nc.sync.dma_start(b3_sb, b3.rearrange("(a o) -> a o", a=1))
            w3_sb = consts.tile([P, KH], f32r)
            nc.sync.dma_start(w3_sb, w3.rearrange("(k p) o -> p (k o)", p=P))

            # ---- hidden / weight streaming, explicitly ordered so the
            # pooling pipeline is never starved: hidden batches flow
            # continuously; w1 lands before fc1 of chunk 0 needs it and w2
            # before fc2 of chunk 0. HF = 770: cols 768:770 are constant 1.0
            # so the pooling matmul also produces per-segment counts. ----
            hbs = [None] * BL
            w1_sb = w2_sb = None

            def load_hb(b):
                hb = hpool.tile([P, KT, HF], f32r, tag=f"hb{b % 5}", name=f"hb{b}")
                nc.sync.dma_start(hb, hid[b].rearrange("(k p) h -> p k h", p=P))
                hbs[b] = hb

            load_hb(0)
            load_hb(1)
            w1_sb = wpool.tile([P, KH, H], f32r, tag="w1")
            nc.sync.dma_start(w1_sb, w1.rearrange("(k p) j -> p k j", p=P))
            load_hb(2)
            load_hb(3)
            load_hb(4)
            w2_sb = wpool.tile([P, KH, H], f32r, tag="w2")
            nc.sync.dma_start(w2_sb, w2.rearrange("(k p) j -> p k j", p=P))
            load_hb(5)
            load_hb(6)
            load_hb(7)

            xts = [xtpool.tile([P, R], f32r, tag=f"xt{k}", name=f"xt{k}") for k in range(KH)]
            y1s = [ypool.tile([P, R], f32r, tag=f"y1_{m}", name=f"y1_{m}") for m in range(KH)]
            y2s = [ypool.tile([P, R], f32r, tag=f"y2_{m}", name=f"y2_{m}") for m in range(KH)]
            pred = ypool.tile([1, R], f32, tag="pred")

            C0 = 512          # pooling psum chunk 0: cols [0, 512)
            C1 = HF - C0      # chunk 1: cols [512, 770) -- col 768 = counts

            def pool_batch(b):
                hb = hbs[b]
                sidf = small.tile([P, KT], f32, tag="sidf")
                nc.vector.tensor_copy(sidf, sid_sb[:, b, :])
                mts = []
                for k in range(KT):
                    mt = mtpool.tile([P, P], f32r, tag="mt")
                    nc.vector.tensor_tensor(
                        mt,
                        iota_sb,
                        sidf[:, k : k + 1].to_broadcast((P, P)),
                        OP.is_equal,
                    )
                    mts.append(mt)
                pp0 = ps.tile([P, C0], f32, tag="ps")
                pp1 = ps.tile([P, C1], f32, tag="ps")
                for k in range(KT):
                    nc.tensor.matmul(
                        pp0, lhsT=mts[k], rhs=hb[:, k, 0:C0],
                        start=(k == 0), stop=(k == KT - 1),
                    )
                for k in range(KT):
                    nc.tensor.matmul(
                        pp1, lhsT=mts[k], rhs=hb[:, k, C0:HF],
                        start=(k == 0), stop=(k == KT - 1),
                    )
                inv = small.tile([P, 1], f32, tag="inv")
                nc.vector.tensor_scalar(inv, pp1[:, H - C0 : H - C0 + 1], 1.0, None, OP.max)
                nc.vector.reciprocal(inv, inv)
                pooled = small.tile([P, H], f32, tag="pooled")
                nc.vector.tensor_tensor(
                    pooled[:, 0:C0], pp0, inv[:, 0:1].to_broadcast((P, C0)), OP.mult
                )
                nc.vector.tensor_tensor(
                    pooled[:, C0:H], pp1[:, 0 : H - C0],
                    inv[:, 0:1].to_broadcast((P, H - C0)), OP.mult,
                )
                for m in range(KH):
                    trp = ps.tile([P, P], f32, tag="ps")
                    nc.tensor.transpose(trp, pooled[:, m * P : (m + 1) * P], ident_sb)
                    nc.vector.tensor_copy(xts[m][:, b * S : (b + 1) * S], trp)

            def fc(w_sb, b_sb, xs, outs, rc, func):
                for m in range(KH):
                    pt = ps.tile([P, RC], f32, tag="ps")
                    for k in range(KH):
                        nc.tensor.matmul(
                            pt,
                            lhsT=w_sb[:, k, m * P : (m + 1) * P],
                            rhs=xs[k][:, rc * RC : (rc + 1) * RC],
                            start=(k == 0),
                            stop=(k == KH - 1),
                        )
                    nc.scalar.activation(
                        outs[m][:, rc * RC : (rc + 1) * RC],
                        pt,
                        func,
                        bias=b_sb[:, m : m + 1],
                    )

            def fc3(rc):
                pt = ps.tile([1, RC], f32, tag="ps")
                for k in range(KH):
                    nc.tensor.matmul(
                        pt,
                        lhsT=w3_sb[:, k : k + 1],
                        rhs=y2s[k][:, rc * RC : (rc + 1) * RC],
                        start=(k == 0),
                        stop=(k == KH - 1),
                    )
                nc.scalar.activation(
                    pred[:, rc * RC : (rc + 1) * RC],
                    pt,
                    mybir.ActivationFunctionType.Sigmoid,
                    bias=b3_sb,
                )

            FT = mybir.ActivationFunctionType
            gelu = FT.Gelu if act_func is None else act_func
            for i in range(NRC):
                if i >= 1:
                    fc(w2_sb, b2_sb, y1s, y2s, i - 1, gelu)
                pool_batch(2 * i)
                pool_batch(2 * i + 1)
                fc(w1_sb, b1_sb, xts, y1s, i, gelu)
            fc(w2_sb, b2_sb, y1s, y2s, NRC - 1, gelu)
            for i in range(NRC):
                fc3(i)
            nc.sync.dma_start(out.rearrange("b s -> (b s)"), pred)

    nc.compile()
    return nc


def _get_program():
    if "nc" not in _CACHE:
        _CACHE["nc"] = _build_program()
    return _CACHE["nc"]


def _consts():
    iota = np.broadcast_to(np.arange(P, dtype=np.float32), (P, P)).copy()
    ident = np.eye(P, dtype=np.float32)
    return iota, ident


def make_in_maps(hidden, statements_ids, w1, b1, w2, b2, w3, b3):
    iota, ident = _consts()
    hidden = np.asarray(hidden, dtype=np.float32)
    pad = np.ones((*hidden.shape[:2], HF - H), dtype=np.float32)
    hidden = np.ascontiguousarray(np.concatenate([hidden, pad], axis=-1))
    sid = np.ascontiguousarray(np.asarray(statements_ids, dtype=np.int32))
    in_maps = []
    for c in range(N_CORES):
        in_maps.append(
            {
                "hidden": hidden[c * BL : (c + 1) * BL],
                "sid": sid[c * BL : (c + 1) * BL],
                "w1": np.asarray(w1, dtype=np.float32),
                "w2": np.asarray(w2, dtype=np.float32),
                "w3": np.asarray(w3, dtype=np.float32),
                "b1": np.asarray(b1, dtype=np.float32),
                "b2": np.asarray(b2, dtype=np.float32),
                "b3": np.asarray(b3, dtype=np.float32),
                "iota": iota,
                "ident": ident,
            }
        )
    return in_maps


def kernel(hidden, statements_ids, w1, b1, w2, b2, w3, b3, **kwargs):
    nc = _get_program()
    in_maps = make_in_maps(hidden, statements_ids, w1, b1, w2, b2, w3, b3)
    trace = bool(int(os.environ.get("KERNEL_TRACE", "0")))
    res = bass_utils.run_bass_kernel_spmd(
        nc, in_maps, core_ids=list(range(N_CORES)), trace=trace
    )
    _CACHE["last_results"] = res
    out = np.concatenate([res.results[c]["out"] for c in range(N_CORES)], axis=0)
    return out.astype(np.float32)


# revision 12
# speedup vs baseline: 1.0980x; 1.0980x over previous
"""Trainium2 Bass kernel: per-batch segment-mean pooling + 3-layer MLP.

Reference computation (B=64, T=512, H=768, S=128):
  pooled[b,s,:] = mean over t of hidden[b,t,:] where statements_ids[b,t]==s
  x = gelu(pooled @ w1 + b1); x = gelu(x @ w2 + b2)
  out[b,s] = sigmoid(x @ w3 + b3)

Distribution: data-parallel over batch across 8 NeuronCores (8 batches per
core); MLP weights replicated.

Per-core algorithm (all matmuls on PE at fp32r 1 cycle/row):
  - Build the one-hot matrix MT[t,s] = (sid[t]==s) on DVE via
    tensor_scalar(is_equal) against an iota constant.
  - counts = MT.T @ ones            (PE)        -> inv = 1/max(counts,1) (DVE)
  - pooled_sums = MT.T @ hidden[b]  (PE, [S,H]) -> pooled = sums*inv     (DVE)
  - X^T tiles via PE transpose (pooled is [S,H] but the MLP wants [H, rows])
  - MLP batched over all 8 local batches: rows = 8*128 = 1024 moving dim,
    weights stationary; gelu/sigmoid + bias fused on ACT.
"""

import os
import sys

sys.path.insert(0, "/opt/trn_rl_repo")

import numpy as np

import concourse.bass as bass
import concourse.mybir as mybir
import concourse.tile as tile
from concourse import bacc, bass_utils

B, T, H, S = 64, 512, 768, 128
N_CORES = 8
BL = B // N_CORES  # local batches per core
P = 128
KT = T // P        # t-tiles per batch
KH = H // P        # h-tiles
R = BL * S         # MLP rows per core
RC = 2 * S         # moving-dim chunk (2 batches) -- >=256 keeps fp32r at 1 cyc/row
NRC = R // RC
HF = H + 2         # hidden padded with 2 constant 1.0 columns (counts trick)

_CACHE: dict = {}


def _build_program(act_func=None):
    f32, f32r, i32 = mybir.dt.float32, mybir.dt.float32r, mybir.dt.int32
    FT = mybir.ActivationFunctionType
    OP = mybir.AluOpType

    nc = bacc.Bacc("TRN2", target_bir_lowering=False, debug=False)
    hid = nc.dram_tensor("hidden", [BL, T, HF], f32r, kind="ExternalInput").ap()
    sid = nc.dram_tensor("sid", [BL, T], i32, kind="ExternalInput").ap()
    w1 = nc.dram_tensor("w1", [H, H], f32r, kind="ExternalInput").ap()
    w2 = nc.dram_tensor("w2", [H, H], f32r, kind="ExternalInput").ap()
    w3 = nc.dram_tensor("w3", [H, 1], f32r, kind="ExternalInput").ap()
    b1 = nc.dram_tensor("b1", [H], f32, kind="ExternalInput").ap()
    b2 = nc.dram_tensor("b2", [H], f32, kind="ExternalInput").ap()
    b3 = nc.dram_tensor("b3", [1], f32, kind="ExternalInput").ap()
    iota = nc.dram_tensor("iota", [P, P], f32, kind="ExternalInput").ap()
    ident = nc.dram_tensor("ident", [P, P], f32, kind="ExternalInput").ap()
    out = nc.dram_tensor("out", [BL, S], f32, kind="ExternalOutput").ap()

    with tile.TileContext(nc) as tc:
        with (
            tc.tile_pool(name="consts", bufs=1) as consts,
            tc.tile_pool(name="wpool", bufs=1) as wpool,
            tc.tile_pool(name="hpool", bufs=1) as hpool,
            tc.tile_pool(name="mtpool", bufs=8) as mtpool,
            tc.tile_pool(name="small", bufs=3) as small,
            tc.tile_pool(name="xtpool", bufs=1) as xtpool,
            tc.tile_pool(name="ypool", bufs=1) as ypool,
            tc.tile_pool(name="ps", bufs=8, space="PSUM") as ps,
        ):
            # ---- small constants via the gpsimd (SWDGE) DMA path so the
            # sync/HWDGE queues start streaming hidden immediately ----
            iota_sb = consts.tile([P, P], f32)
            nc.gpsimd.dma_start(iota_sb, iota)
            ident_sb = consts.tile([P, P], f32r)
            nc.gpsimd.dma_start(ident_sb, ident)
            sid_sb = consts.tile([P, BL, KT], i32)
            nc.gpsimd.dma_start(sid_sb, sid.rearrange("b (k p) -> p b k", p=P))
            b1_sb = consts.tile([P, KH], f32)
            nc.gpsimd.dma_start(b1_sb, b1.rearrange("(m p) -> p m", p=P))
            b2_sb = consts.tile([P, KH], f32)
            nc.gpsimd.dma_start(b2_sb, b2.rearrange("(m p) -> p m", p=P))
            b3_sb = consts.tile([1, 1], f32)
            nc.gpsimd.dma_start(b3_sb, b3.rearrange("(a o) -> a o", a=1))
            w3_sb = consts.tile([P, KH], f32r)
            nc.gpsimd.dma_start(w3_sb, w3.rearrange("(k p) o -> p (k o)", p=P))

            # ---- hidden + weight streaming on sync/HWDGE, ordered to match
            # the compute pipeline: hidden batches pace the pooling; weight
            # k-tiles trickle between batches so fc1/fc2 unlock per-k ----
            hbs = [[None] * KT for _ in range(BL)]
            w1ks = [None] * KH
            w2ks = [None] * KH

            def load_hb(b):
                # one tile per (batch, k-chunk) so pooling matmuls unlock as
                # soon as their 128-token slice lands, not the whole batch
                for k in range(KT):
                    t = hpool.tile([P, HF], f32r, tag=f"hb{b % 5}k{k}", name=f"hb{b}k{k}")
                    nc.sync.dma_start(
                        t, hid[b, k * P : (k + 1) * P, :].rearrange("p h -> p h")
                    )
                    hbs[b][k] = t

            def load_w(ws, wdram, k, nm):
                ws[k] = wpool.tile([P, H], f32r, tag=f"{nm}{k}", name=f"{nm}{k}")
                nc.sync.dma_start(ws[k], wdram[k * P : (k + 1) * P, :])

            load_hb(0)
            load_hb(1)
            for k in range(KH):
                load_w(w1ks, w1, k, "w1k")
            load_hb(2)
            load_hb(3)
            for k in range(3):
                load_w(w2ks, w2, k, "w2k")
            load_hb(4)
            for k in range(3, KH):
                load_w(w2ks, w2, k, "w2k")
            load_hb(5)
            load_hb(6)
            load_hb(7)

            xts = [xtpool.tile([P, R], f32r, tag=f"xt{k}", name=f"xt{k}") for k in range(KH)]
            y1s = [ypool.tile([P, R], f32r, tag=f"y1_{m}", name=f"y1_{m}") for m in range(KH)]
            y2s = [ypool.tile([P, R], f32r, tag=f"y2_{m}", name=f"y2_{m}") for m in range(KH)]
            pred = ypool.tile([1, R], f32, tag="pred")

            C0 = 512          # pooling psum chunk 0: cols [0, 512)
            C1 = HF - C0      # chunk 1: cols [512, 770) -- col 768 = counts

            def pool_batch(b):
                sidf = small.tile([P, KT], f32, tag="sidf")
                nc.vector.tensor_copy(sidf, sid_sb[:, b, :])
                mts = []
                for k in range(KT):
                    mt = mtpool.tile([P, P], f32r, tag="mt")
                    nc.vector.tensor_tensor(
                        mt,
                        iota_sb,
                        sidf[:, k : k + 1].to_broadcast((P, P)),
                        OP.is_equal,
                    )
                    mts.append(mt)
                pp0 = ps.tile([P, C0], f32, tag="ps")
                pp1 = ps.tile([P, C1], f32, tag="ps")
                for k in range(KT):
                    nc.tensor.matmul(
                        pp0, lhsT=mts[k], rhs=hbs[b][k][:, 0:C0],
                        start=(k == 0), stop=(k == KT - 1),
                    )
                for k in range(KT):
                    nc.tensor.matmul(
                        pp1, lhsT=mts[k], rhs=hbs[b][k][:, C0:HF],
                        start=(k == 0), stop=(k == KT - 1),
                    )
                inv = small.tile([P, 1], f32, tag="inv")
                nc.vector.tensor_scalar(inv, pp1[:, H - C0 : H - C0 + 1], 1.0, None, OP.max)
                nc.vector.reciprocal(inv, inv)
                # normalize on ACT (Copy with per-partition scale) to keep the
                # DVE off the pool critical chain
                pooled = small.tile([P, H], f32r, tag="pooled")
                nc.scalar.activation(
                    pooled[:, 0:C0], pp0, FT.Copy, bias=0.0, scale=inv[:, 0:1]
                )
                nc.scalar.activation(
                    pooled[:, C0:H], pp1[:, 0 : H - C0], FT.Copy, bias=0.0,
                    scale=inv[:, 0:1],
                )
                for m in range(KH):
                    trp = ps.tile([P, P], f32r, tag="ps")
                    nc.tensor.transpose(trp, pooled[:, m * P : (m + 1) * P], ident_sb)
                    nc.vector.tensor_copy(xts[m][:, b * S : (b + 1) * S], trp)

            def fc(wks, b_sb, xs, outs, rc, func):
                for m in range(KH):
                    pt = ps.tile([P, RC], f32, tag="ps")
                    for k in range(KH):
                        nc.tensor.matmul(
                            pt,
                            lhsT=wks[k][:, m * P : (m + 1) * P],
                            rhs=xs[k][:, rc * RC : (rc + 1) * RC],
                            start=(k == 0),
                            stop=(k == KH - 1),
                        )
                    nc.scalar.activation(
                        outs[m][:, rc * RC : (rc + 1) * RC],
                        pt,
                        func,
                        bias=b_sb[:, m : m + 1],
                    )

            def fc3(rc):
                pt = ps.tile([1, RC], f32, tag="ps")
                for k in range(KH):
                    nc.tensor.matmul(
                        pt,
                        lhsT=w3_sb[:, k : k + 1],
                        rhs=y2s[k][:, rc * RC : (rc + 1) * RC],
                        start=(k == 0),
                        stop=(k == KH - 1),
                    )
                nc.scalar.activation(
                    pred[:, rc * RC : (rc + 1) * RC],
                    pt,
                    mybir.ActivationFunctionType.Sigmoid,
                    bias=b3_sb,
                )

            FT = mybir.ActivationFunctionType
            gelu = FT.Gelu if act_func is None else act_func
            pool_batch(0)
            pool_batch(1)
            fc(w1ks, b1_sb, xts, y1s, 0, gelu)
            pool_batch(2)
            pool_batch(3)
            fc(w1ks, b1_sb, xts, y1s, 1, gelu)
            pool_batch(4)
            fc(w2ks, b2_sb, y1s, y2s, 0, gelu)
            fc3(0)
            pool_batch(5)
            fc(w1ks, b1_sb, xts, y1s, 2, gelu)
            fc(w2ks, b2_sb, y1s, y2s, 1, gelu)
            fc3(1)
            pool_batch(6)
            fc(w2ks, b2_sb, y1s, y2s, 2, gelu)
            fc3(2)
            pool_batch(7)
            fc(w1ks, b1_sb, xts, y1s, 3, gelu)
            fc(w2ks, b2_sb, y1s, y2s, 3, gelu)
            fc3(3)
            nc.sync.dma_start(out.rearrange("b s -> (b s)"), pred)

    nc.compile()
    return nc


def _get_program():
    if "nc" not in _CACHE:
        _CACHE["nc"] = _build_program()
    return _CACHE["nc"]


def _consts():
    iota = np.broadcast_to(np.arange(P, dtype=np.float32), (P, P)).copy()
    ident = np.eye(P, dtype=np.float32)
    return iota, ident


def make_in_maps(hidden, statements_ids, w1, b1, w2, b2, w3, b3):
    iota, ident = _consts()
    hidden = np.asarray(hidden, dtype=np.float32)
    pad = np.ones((*hidden.shape[:2], HF - H), dtype=np.float32)
    hidden = np.ascontiguousarray(np.concatenate([hidden, pad], axis=-1))
    sid = np.ascontiguousarray(np.asarray(statements_ids, dtype=np.int32))
    in_maps = []
    for c in range(N_CORES):
        in_maps.append(
            {
                "hidden": hidden[c * BL : (c + 1) * BL],
                "sid": sid[c * BL : (c + 1) * BL],
                "w1": np.asarray(w1, dtype=np.float32),
                "w2": np.asarray(w2, dtype=np.float32),
                "w3": np.asarray(w3, dtype=np.float32),
                "b1": np.asarray(b1, dtype=np.float32),
                "b2": np.asarray(b2, dtype=np.float32),
                "b3": np.asarray(b3, dtype=np.float32),
                "iota": iota,
                "ident": ident,
            }
        )
    return in_maps


def kernel(hidden, statements_ids, w1, b1, w2, b2, w3, b3, **kwargs):
    nc = _get_program()
    in_maps = make_in_maps(hidden, statements_ids, w1, b1, w2, b2, w3, b3)
    trace = bool(int(os.environ.get("KERNEL_TRACE", "0")))
    res = bass_utils.run_bass_kernel_spmd(
        nc, in_maps, core_ids=list(range(N_CORES)), trace=trace
    )
    _CACHE["last_results"] = res
    out = np.concatenate([res.results[c]["out"] for c in range(N_CORES)], axis=0)
    return out.astype(np.float32)


# revision 14
# speedup vs baseline: 1.1934x; 1.0868x over previous
"""Trainium2 Bass kernel: per-batch segment-mean pooling + 3-layer MLP.

Reference computation (B=64, T=512, H=768, S=128):
  pooled[b,s,:] = mean over t of hidden[b,t,:] where statements_ids[b,t]==s
  x = gelu(pooled @ w1 + b1); x = gelu(x @ w2 + b2)
  out[b,s] = sigmoid(x @ w3 + b3)

Distribution: data-parallel over batch across 8 NeuronCores (8 batches per
core); MLP weights replicated.

Per-core algorithm (all matmuls on PE at fp32r 1 cycle/row):
  - Build the one-hot matrix MT[t,s] = (sid[t]==s) on DVE via
    tensor_scalar(is_equal) against an iota constant.
  - counts = MT.T @ ones            (PE)        -> inv = 1/max(counts,1) (DVE)
  - pooled_sums = MT.T @ hidden[b]  (PE, [S,H]) -> pooled = sums*inv     (DVE)
  - X^T tiles via PE transpose (pooled is [S,H] but the MLP wants [H, rows])
  - MLP batched over all 8 local batches: rows = 8*128 = 1024 moving dim,
    weights stationary; gelu/sigmoid + bias fused on ACT.
"""

import os
import sys

sys.path.insert(0, "/opt/trn_rl_repo")

import numpy as np

import concourse.bass as bass
import concourse.mybir as mybir
import concourse.tile as tile
from concourse import bacc, bass_utils

B, T, H, S = 64, 512, 768, 128
N_CORES = 8
BL = B // N_CORES  # local batches per core
P = 128
KT = T // P        # t-tiles per batch
KH = H // P        # h-tiles
R = BL * S         # MLP rows per core
RC = 2 * S         # moving-dim chunk (2 batches) -- >=256 keeps fp32r at 1 cyc/row
NRC = R // RC
HF = H + 2         # hidden padded with 2 constant 1.0 columns (counts trick)

_CACHE: dict = {}


def _build_program(act_func=None):
    f32, f32r, i32 = mybir.dt.float32, mybir.dt.float32r, mybir.dt.int32
    FT = mybir.ActivationFunctionType
    OP = mybir.AluOpType

    nc = bacc.Bacc("TRN2", target_bir_lowering=False, debug=False)
    hid = nc.dram_tensor("hidden", [BL, T, HF], f32r, kind="ExternalInput").ap()
    sid = nc.dram_tensor("sid", [BL, T], i32, kind="ExternalInput").ap()
    w1 = nc.dram_tensor("w1", [H, H], f32r, kind="ExternalInput").ap()
    w2 = nc.dram_tensor("w2", [H, H], f32r, kind="ExternalInput").ap()
    w3 = nc.dram_tensor("w3", [H, 1], f32r, kind="ExternalInput").ap()
    b1 = nc.dram_tensor("b1", [H], f32, kind="ExternalInput").ap()
    b2 = nc.dram_tensor("b2", [H], f32, kind="ExternalInput").ap()
    b3 = nc.dram_tensor("b3", [1], f32, kind="ExternalInput").ap()
    iota = nc.dram_tensor("iota", [P, P], f32, kind="ExternalInput").ap()
    ident = nc.dram_tensor("ident", [P, P], f32r, kind="ExternalInput").ap()
    out = nc.dram_tensor("out", [BL, S], f32, kind="ExternalOutput").ap()

    with tile.TileContext(nc) as tc:
        with (
            tc.tile_pool(name="consts", bufs=1) as consts,
            tc.tile_pool(name="wpool", bufs=1) as wpool,
            tc.tile_pool(name="hpool", bufs=1) as hpool,
            tc.tile_pool(name="mtpool", bufs=8) as mtpool,
            tc.tile_pool(name="small", bufs=3) as small,
            tc.tile_pool(name="xtpool", bufs=1) as xtpool,
            tc.tile_pool(name="ypool", bufs=1) as ypool,
            tc.tile_pool(name="ps", bufs=8, space="PSUM") as ps,
        ):
            # ---- small constants first on the sync/HWDGE path (~150 KB,
            # well under 1us of stream; the SWDGE path takes ~13us to spin
            # up, which would stall the first is_equal) ----
            iota_sb = consts.tile([P, P], f32)
            nc.sync.dma_start(iota_sb, iota)
            ident_sb = consts.tile([P, P], f32r)
            nc.sync.dma_start(ident_sb, ident)
            sid_sb = consts.tile([P, BL, KT], i32)
            nc.sync.dma_start(sid_sb, sid.rearrange("b (k p) -> p b k", p=P))
            b1_sb = consts.tile([P, KH], f32)
            nc.sync.dma_start(b1_sb, b1.rearrange("(m p) -> p m", p=P))
            b2_sb = consts.tile([P, KH], f32)
            nc.sync.dma_start(b2_sb, b2.rearrange("(m p) -> p m", p=P))
            b3_sb = consts.tile([1, 1], f32)
            nc.sync.dma_start(b3_sb, b3.rearrange("(a o) -> a o", a=1))
            w3_sb = consts.tile([P, KH], f32r)
            nc.sync.dma_start(w3_sb, w3.rearrange("(k p) o -> p (k o)", p=P))

            # ---- hidden + weight streaming on sync/HWDGE, ordered to match
            # the compute pipeline: hidden batches pace the pooling; weight
            # k-tiles trickle between batches so fc1/fc2 unlock per-k ----
            hbs = [[None] * KT for _ in range(BL)]
            w1ks = [None] * KH
            w2ks = [None] * KH

            def load_hb(b):
                # one tile per (batch, k-chunk) so pooling matmuls unlock as
                # soon as their 128-token slice lands, not the whole batch
                for k in range(KT):
                    t = hpool.tile([P, HF], f32r, tag=f"hb{b % 5}k{k}", name=f"hb{b}k{k}")
                    nc.sync.dma_start(
                        t, hid[b, k * P : (k + 1) * P, :]
                    )
                    hbs[b][k] = t

            def load_w(ws, wdram, k, nm):
                ws[k] = wpool.tile([P, H], f32r, tag=f"{nm}{k}", name=f"{nm}{k}")
                nc.sync.dma_start(ws[k], wdram[k * P : (k + 1) * P, :])

            load_hb(0)
            load_hb(1)
            for k in range(KH):
                load_w(w1ks, w1, k, "w1k")
            load_hb(2)
            load_hb(3)
            for k in range(3):
                load_w(w2ks, w2, k, "w2k")
            load_hb(4)
            for k in range(3, KH):
                load_w(w2ks, w2, k, "w2k")
            load_hb(5)
            load_hb(6)
            load_hb(7)

            xts = [xtpool.tile([P, R], f32r, tag=f"xt{k}", name=f"xt{k}") for k in range(KH)]
            y1s = [ypool.tile([P, R], f32r, tag=f"y1_{m}", name=f"y1_{m}") for m in range(KH)]
            y2s = [ypool.tile([P, R], f32r, tag=f"y2_{m}", name=f"y2_{m}") for m in range(KH)]
            pred = ypool.tile([1, R], f32, tag="pred")

            C0 = 512          # pooling psum chunk 0: cols [0, 512)
            C1 = HF - C0      # chunk 1: cols [512, 770) -- col 768 = counts

            def pool_batch(b):
                sidf = small.tile([P, KT], f32, tag="sidf")
                nc.vector.tensor_copy(sidf, sid_sb[:, b, :])
                mts = []
                for k in range(KT):
                    mt = mtpool.tile([P, P], f32r, tag="mt")
                    nc.vector.tensor_tensor(
                        mt,
                        iota_sb,
                        sidf[:, k : k + 1].to_broadcast((P, P)),
                        OP.is_equal,
                    )
                    mts.append(mt)
                pp0 = ps.tile([P, C0], f32, tag="ps")
                pp1 = ps.tile([P, C1], f32, tag="ps")
                for k in range(KT):
                    nc.tensor.matmul(
                        pp0, lhsT=mts[k], rhs=hbs[b][k][:, 0:C0],
                        start=(k == 0), stop=(k == KT - 1),
                    )
                for k in range(KT):
                    nc.tensor.matmul(
                        pp1, lhsT=mts[k], rhs=hbs[b][k][:, C0:HF],
                        start=(k == 0), stop=(k == KT - 1),
                    )
                inv = small.tile([P, 1], f32, tag="inv")
                nc.vector.tensor_scalar(inv, pp1[:, H - C0 : H - C0 + 1], 1.0, None, OP.max)
                nc.vector.reciprocal(inv, inv)
                pooled = small.tile([P, H], f32r, tag="pooled")
                nc.vector.tensor_tensor(
                    pooled[:, 0:C0], pp0, inv[:, 0:1].to_broadcast((P, C0)), OP.mult
                )
                nc.vector.tensor_tensor(
                    pooled[:, C0:H], pp1[:, 0 : H - C0],
                    inv[:, 0:1].to_broadcast((P, H - C0)), OP.mult,
                )
                for m in range(KH):
                    trp = ps.tile([P, P], f32r, tag="ps")
                    nc.tensor.transpose(trp, pooled[:, m * P : (m + 1) * P], ident_sb)
                    nc.vector.tensor_copy(xts[m][:, b * S : (b + 1) * S], trp)

            def fc(wks, b_sb, xs, outs, rc, func):
                for m in range(KH):
                    pt = ps.tile([P, RC], f32, tag="ps")
                    for k in range(KH):
                        nc.tensor.matmul(
                            pt,
                            lhsT=wks[k][:, m * P : (m + 1) * P],
                            rhs=xs[k][:, rc * RC : (rc + 1) * RC],
                            start=(k == 0),
                            stop=(k == KH - 1),
                        )
                    nc.scalar.activation(
                        outs[m][:, rc * RC : (rc + 1) * RC],
                        pt,
                        func,
                        bias=b_sb[:, m : m + 1],
                    )

            def fc3(rc):
                pt = ps.tile([1, RC], f32, tag="ps")
                for k in range(KH):
                    nc.tensor.matmul(
                        pt,
                        lhsT=w3_sb[:, k : k + 1],
                        rhs=y2s[k][:, rc * RC : (rc + 1) * RC],
                        start=(k == 0),
                        stop=(k == KH - 1),
                    )
                nc.scalar.activation(
                    pred[:, rc * RC : (rc + 1) * RC],
                    pt,
                    mybir.ActivationFunctionType.Sigmoid,
                    bias=b3_sb,
                )

            FT = mybir.ActivationFunctionType
            gelu = FT.Gelu if act_func is None else act_func
            pool_batch(0)
            pool_batch(1)
            fc(w1ks, b1_sb, xts, y1s, 0, gelu)
            pool_batch(2)
            pool_batch(3)
            fc(w1ks, b1_sb, xts, y1s, 1, gelu)
            pool_batch(4)
            fc(w2ks, b2_sb, y1s, y2s, 0, gelu)
            fc3(0)
            pool_batch(5)
            fc(w1ks, b1_sb, xts, y1s, 2, gelu)
            fc(w2ks, b2_sb, y1s, y2s, 1, gelu)
            fc3(1)
            pool_batch(6)
            fc(w2ks, b2_sb, y1s, y2s, 2, gelu)
            fc3(2)
            pool_batch(7)
            fc(w1ks, b1_sb, xts, y1s, 3, gelu)
            fc(w2ks, b2_sb, y1s, y2s, 3, gelu)
            fc3(3)
            nc.sync.dma_start(out.rearrange("b s -> (b s)"), pred)

    nc.compile()
    return nc


def _get_program():
    if "nc" not in _CACHE:
        _CACHE["nc"] = _build_program()
    return _CACHE["nc"]


def _consts():
    iota = np.broadcast_to(np.arange(P, dtype=np.float32), (P, P)).copy()
    ident = np.eye(P, dtype=np.float32)
    return iota, ident


def make_in_maps(hidden, statements_ids, w1, b1, w2, b2, w3, b3):
    iota, ident = _consts()
    hidden = np.asarray(hidden, dtype=np.float32)
    pad = np.ones((*hidden.shape[:2], HF - H), dtype=np.float32)
    hidden = np.ascontiguousarray(np.concatenate([hidden, pad], axis=-1))
    sid = np.ascontiguousarray(np.asarray(statements_ids, dtype=np.int32))
    in_maps = []
    for c in range(N_CORES):
        in_maps.append(
            {
                "hidden": hidden[c * BL : (c + 1) * BL],
                "sid": sid[c * BL : (c + 1) * BL],
                "w1": np.asarray(w1, dtype=np.float32),
                "w2": np.asarray(w2, dtype=np.float32),
                "w3": np.asarray(w3, dtype=np.float32),
                "b1": np.asarray(b1, dtype=np.float32),
                "b2": np.asarray(b2, dtype=np.float32),
                "b3": np.asarray(b3, dtype=np.float32),
                "iota": iota,
                "ident": ident,
            }
        )
    return in_maps


def kernel(hidden, statements_ids, w1, b1, w2, b2, w3, b3, **kwargs):
    nc = _get_program()
    in_maps = make_in_maps(hidden, statements_ids, w1, b1, w2, b2, w3, b3)
    trace = bool(int(os.environ.get("KERNEL_TRACE", "0")))
    res = bass_utils.run_bass_kernel_spmd(
        nc, in_maps, core_ids=list(range(N_CORES)), trace=trace
    )
    _CACHE["last_results"] = res
    out = np.concatenate([res.results[c]["out"] for c in range(N_CORES)], axis=0)
    return out.astype(np.float32)


# revision 17
# speedup vs baseline: 1.2126x; 1.0161x over previous
"""Trainium2 Bass kernel: per-batch segment-mean pooling + 3-layer MLP.

Reference computation (B=64, T=512, H=768, S=128):
  pooled[b,s,:] = mean over t of hidden[b,t,:] where statements_ids[b,t]==s
  x = gelu(pooled @ w1 + b1); x = gelu(x @ w2 + b2)
  out[b,s] = sigmoid(x @ w3 + b3)

Distribution: data-parallel over batch across 8 NeuronCores (8 batches per
core); MLP weights replicated.

Per-core algorithm (all matmuls on PE at fp32r 1 cycle/row):
  - Build the one-hot matrix MT[t,s] = (sid[t]==s) on DVE via
    tensor_scalar(is_equal) against an iota constant.
  - counts = MT.T @ ones            (PE)        -> inv = 1/max(counts,1) (DVE)
  - pooled_sums = MT.T @ hidden[b]  (PE, [S,H]) -> pooled = sums*inv     (DVE)
  - X^T tiles via PE transpose (pooled is [S,H] but the MLP wants [H, rows])
  - MLP batched over all 8 local batches: rows = 8*128 = 1024 moving dim,
    weights stationary; gelu/sigmoid + bias fused on ACT.
"""

import os
import sys

sys.path.insert(0, "/opt/trn_rl_repo")

import numpy as np

import concourse.bass as bass
import concourse.mybir as mybir
import concourse.tile as tile
from concourse import bacc, bass_utils

B, T, H, S = 64, 512, 768, 128
N_CORES = 8
BL = B // N_CORES  # local batches per core
P = 128
KT = T // P        # t-tiles per batch
KH = H // P        # h-tiles
R = BL * S         # MLP rows per core
RC = 2 * S         # moving-dim chunk (2 batches) -- >=256 keeps fp32r at 1 cyc/row
NRC = R // RC
HF = H + 2         # hidden padded with 2 constant 1.0 columns (counts trick)
CR_COLS = 134      # f32r packed consts (matmul operands): ident | w3
CF_COLS = 173      # f32 packed consts: iota | sid-bits | b1 | b2 | b3

_CACHE: dict = {}


def _build_program(act_func=None):
    f32, f32r, i32 = mybir.dt.float32, mybir.dt.float32r, mybir.dt.int32
    FT = mybir.ActivationFunctionType
    OP = mybir.AluOpType

    nc = bacc.Bacc("TRN2", target_bir_lowering=False, debug=False)
    hid = nc.dram_tensor("hidden", [BL, T, HF], f32r, kind="ExternalInput").ap()
    w1 = nc.dram_tensor("w1", [H, H], f32r, kind="ExternalInput").ap()
    w2 = nc.dram_tensor("w2", [H, H], f32r, kind="ExternalInput").ap()
    cpack_r = nc.dram_tensor("cpack_r", [P, CR_COLS], f32r, kind="ExternalInput").ap()
    cpack_f = nc.dram_tensor("cpack_f", [P, CF_COLS], f32, kind="ExternalInput").ap()
    out = nc.dram_tensor("out", [BL, S], f32, kind="ExternalOutput").ap()

    with tile.TileContext(nc) as tc:
        with (
            tc.tile_pool(name="consts", bufs=1) as consts,
            tc.tile_pool(name="wpool", bufs=1) as wpool,
            tc.tile_pool(name="hpool", bufs=1) as hpool,
            tc.tile_pool(name="mtpool", bufs=8) as mtpool,
            tc.tile_pool(name="small", bufs=3) as small,
            tc.tile_pool(name="xtpool", bufs=1) as xtpool,
            tc.tile_pool(name="ypool", bufs=1) as ypool,
            tc.tile_pool(name="ps", bufs=8, space="PSUM") as ps,
        ):
            # ---- all small constants arrive in ONE packed DMA (single
            # 1.2KB line per partition) so the hidden stream starts at once ----
            cpr_sb = consts.tile([P, CR_COLS], f32r)
            nc.sync.dma_start(cpr_sb, cpack_r)
            cpf_sb = consts.tile([P, CF_COLS], f32)
            nc.sync.dma_start(cpf_sb, cpack_f)
            ident_sb = cpr_sb[:, 0:P]
            w3_sb = cpr_sb[:, P : P + KH]
            iota_sb = cpf_sb[:, 0:P]
            sid_sb = cpf_sb[:, P : P + BL * KT].bitcast(i32)
            b1_sb = cpf_sb[:, 160:166]
            b2_sb = cpf_sb[:, 166:172]
            b3_sb = cpf_sb[0:1, 172:173]

            # ---- hidden + weight streaming on sync/HWDGE, ordered to match
            # the compute pipeline: hidden batches pace the pooling; weight
            # k-tiles trickle between batches so fc1/fc2 unlock per-k ----
            hbs = [[None] * KT for _ in range(BL)]
            w1ks = [None] * KH
            w2ks = [None] * KH

            def load_hb(b):
                # one tile per (batch, k-chunk) so pooling matmuls unlock as
                # soon as their 128-token slice lands, not the whole batch
                for k in range(KT):
                    t = hpool.tile([P, HF], f32r, tag=f"hb{b % 5}k{k}", name=f"hb{b}k{k}")
                    nc.sync.dma_start(
                        t, hid[b, k * P : (k + 1) * P, :]
                    )
                    hbs[b][k] = t

            def load_w(ws, wdram, k, nm):
                ws[k] = wpool.tile([P, H], f32r, tag=f"{nm}{k}", name=f"{nm}{k}")
                nc.sync.dma_start(ws[k], wdram[k * P : (k + 1) * P, :])

            load_hb(0)
            for k in range(3):
                load_w(w1ks, w1, k, "w1k")
            load_hb(1)
            for k in range(3, KH):
                load_w(w1ks, w1, k, "w1k")
            load_hb(2)
            load_hb(3)
            for k in range(KH):
                load_w(w2ks, w2, k, "w2k")
            load_hb(4)
            load_hb(5)
            load_hb(6)
            load_hb(7)

            xts = [xtpool.tile([P, R], f32r, tag=f"xt{k}", name=f"xt{k}") for k in range(KH)]
            y1s = [ypool.tile([P, R], f32r, tag=f"y1_{m}", name=f"y1_{m}") for m in range(KH)]
            y2s = [ypool.tile([P, R], f32r, tag=f"y2_{m}", name=f"y2_{m}") for m in range(KH)]
            pred = ypool.tile([1, R], f32, tag="pred")

            C0 = 512          # pooling psum chunk 0: cols [0, 512)
            C1 = HF - C0      # chunk 1: cols [512, 770) -- col 768 = counts

            def pool_batch(b):
                sidf = small.tile([P, KT], f32, tag="sidf")
                nc.vector.tensor_copy(sidf, sid_sb[:, b * KT : (b + 1) * KT])
                mts = []
                for k in range(KT):
                    mt = mtpool.tile([P, P], f32r, tag="mt")
                    nc.vector.tensor_tensor(
                        mt,
                        iota_sb,
                        sidf[:, k : k + 1].to_broadcast((P, P)),
                        OP.is_equal,
                    )
                    mts.append(mt)
                pp0 = ps.tile([P, C0], f32, tag="ps")
                pp1 = ps.tile([P, C1], f32, tag="ps")
                for k in range(KT):
                    nc.tensor.matmul(
                        pp0, lhsT=mts[k], rhs=hbs[b][k][:, 0:C0],
                        start=(k == 0), stop=(k == KT - 1),
                    )
                for k in range(KT):
                    nc.tensor.matmul(
                        pp1, lhsT=mts[k], rhs=hbs[b][k][:, C0:HF],
                        start=(k == 0), stop=(k == KT - 1),
                    )
                inv = small.tile([P, 1], f32, tag="inv")
                nc.vector.tensor_scalar(inv, pp1[:, H - C0 : H - C0 + 1], 1.0, None, OP.max)
                nc.vector.reciprocal(inv, inv)
                pooled = small.tile([P, H], f32r, tag="pooled")
                nc.vector.tensor_tensor(
                    pooled[:, 0:C0], pp0, inv[:, 0:1].to_broadcast((P, C0)), OP.mult
                )
                nc.vector.tensor_tensor(
                    pooled[:, C0:H], pp1[:, 0 : H - C0],
                    inv[:, 0:1].to_broadcast((P, H - C0)), OP.mult,
                )
                for m in range(KH):
                    trp = ps.tile([P, P], f32r, tag="ps")
                    nc.tensor.transpose(trp, pooled[:, m * P : (m + 1) * P], ident_sb)
                    nc.vector.tensor_copy(xts[m][:, b * S : (b + 1) * S], trp)

            def fc(wks, b_sb, xs, outs, rc, func):
                for m in range(KH):
                    pt = ps.tile([P, RC], f32, tag="ps")
                    for k in range(KH):
                        nc.tensor.matmul(
                            pt,
                            lhsT=wks[k][:, m * P : (m + 1) * P],
                            rhs=xs[k][:, rc * RC : (rc + 1) * RC],
                            start=(k == 0),
                            stop=(k == KH - 1),
                        )
                    nc.scalar.activation(
                        outs[m][:, rc * RC : (rc + 1) * RC],
                        pt,
                        func,
                        bias=b_sb[:, m : m + 1],
                    )

            def fc3(rc):
                pt = ps.tile([1, RC], f32, tag="ps")
                for k in range(KH):
                    nc.tensor.matmul(
                        pt,
                        lhsT=w3_sb[:, k : k + 1],
                        rhs=y2s[k][:, rc * RC : (rc + 1) * RC],
                        start=(k == 0),
                        stop=(k == KH - 1),
                    )
                nc.scalar.activation(
                    pred[:, rc * RC : (rc + 1) * RC],
                    pt,
                    mybir.ActivationFunctionType.Sigmoid,
                    bias=b3_sb,
                )

            FT = mybir.ActivationFunctionType
            gelu = FT.Gelu if act_func is None else act_func
            pool_batch(0)
            pool_batch(1)
            fc(w1ks, b1_sb, xts, y1s, 0, gelu)
            pool_batch(2)
            pool_batch(3)
            fc(w1ks, b1_sb, xts, y1s, 1, gelu)
            fc(w2ks, b2_sb, y1s, y2s, 0, gelu)
            fc3(0)
            pool_batch(4)
            pool_batch(5)
            fc(w1ks, b1_sb, xts, y1s, 2, gelu)
            fc(w2ks, b2_sb, y1s, y2s, 1, gelu)
            fc3(1)
            pool_batch(6)
            pool_batch(7)
            fc(w1ks, b1_sb, xts, y1s, 3, gelu)
            fc(w2ks, b2_sb, y1s, y2s, 2, gelu)
            fc3(2)
            fc(w2ks, b2_sb, y1s, y2s, 3, gelu)
            fc3(3)
            nc.sync.dma_start(out.rearrange("b s -> (b s)"), pred)

    nc.compile()
    return nc


def _get_program():
    if "nc" not in _CACHE:
        _CACHE["nc"] = _build_program()
    return _CACHE["nc"]


def _cpack(sid_shard, b1, b2, b3, w3):
    """Pack per-core constants into two tensors: f32r (matmul operands,
    the DMA may round these) and plain f32 (bit-exact: iota, sid bits,
    biases)."""
    cr = np.zeros((P, CR_COLS), dtype=np.float32)
    cr[:, 0:P] = np.eye(P, dtype=np.float32)
    cr[:, P : P + KH] = np.asarray(w3, np.float32).reshape(KH, P, 1)[:, :, 0].T
    cf = np.zeros((P, CF_COLS), dtype=np.float32)
    cf[:, 0:P] = np.arange(P, dtype=np.float32)[None, :]
    sid_cols = np.transpose(
        sid_shard.astype(np.int32).reshape(BL, KT, P), (2, 0, 1)
    ).reshape(P, BL * KT)
    cf[:, P : P + BL * KT] = sid_cols.view(np.float32)
    cf[:, 160:166] = np.asarray(b1, np.float32).reshape(KH, P).T
    cf[:, 166:172] = np.asarray(b2, np.float32).reshape(KH, P).T
    cf[0, 172] = np.float32(np.asarray(b3).reshape(-1)[0])
    return cr, cf


def make_in_maps(hidden, statements_ids, w1, b1, w2, b2, w3, b3):
    hidden = np.asarray(hidden, dtype=np.float32)
    pad = np.ones((*hidden.shape[:2], HF - H), dtype=np.float32)
    hidden = np.ascontiguousarray(np.concatenate([hidden, pad], axis=-1))
    sid = np.asarray(statements_ids, dtype=np.int32)
    w1 = np.ascontiguousarray(np.asarray(w1, dtype=np.float32))
    w2 = np.ascontiguousarray(np.asarray(w2, dtype=np.float32))
    in_maps = []
    for c in range(N_CORES):
        cr, cf = _cpack(sid[c * BL : (c + 1) * BL], b1, b2, b3, w3)
        in_maps.append(
            {
                "hidden": hidden[c * BL : (c + 1) * BL],
                "w1": w1,
                "w2": w2,
                "cpack_r": cr,
                "cpack_f": cf,
            }
        )
    return in_maps


def kernel(hidden, statements_ids, w1, b1, w2, b2, w3, b3, **kwargs):
    nc = _get_program()
    in_maps = make_in_maps(hidden, statements_ids, w1, b1, w2, b2, w3, b3)
    trace = bool(int(os.environ.get("KERNEL_TRACE", "0")))
    res = bass_utils.run_bass_kernel_spmd(
        nc, in_maps, core_ids=list(range(N_CORES)), trace=trace
    )
    _CACHE["last_results"] = res
    out = np.concatenate([res.results[c]["out"] for c in range(N_CORES)], axis=0)
    return out.astype(np.float32)
